# revision 54
# baseline (speedup 1.0000x reference)
"""BiLSTM-CRF NLL loss kernel for 8 Trainium2 NeuronCores (pure batch data-parallel).

Self-contained: hardcodes all shapes. Strategy per core (32 of 256 sequences):
  1. Embedding gather (indirect DMA, bf16 table) -> PE transpose -> augmented
     input-projection GEMM gx = W_aug @ [emb; 1; 1-m] (bias + bwd-mask folded in).
  2. Backward LSTM pass (global t descending), then forward pass. Weights
     stationary bf16 matmuls accumulate onto gx preloaded in PSUM; gates via
     tanh only (sigmoid(x) = 0.5*tanh(x/2)+0.5) so one ACT table set serves
     the whole kernel (exp_and_others: tanh+exp+copy).
  3. Bulk fc GEMM -> feats = mask*(W_fc@h) + b_fc -> exp(feats).
  4. CRF forward in scaled probability space: A <- (P @ A) * ef_t with
     P = exp(trans) stationary on PE; renormalize every 6 steps, log(Z)
     summed in bulk at the end (natural_log set, one switch).
  5. Gold score from host-built integer histograms/one-hots via TTR + tiny
     PE reductions. Output: per-core sum of (fwd - gold); host divides by B.
"""

import numpy as np

import concourse.bacc as bacc
import concourse.bass as bass
import concourse.mybir as mybir
import concourse.tile as tile
from concourse import bass_utils

B, T, E, H2, V, NT = 256, 192, 300, 256, 11626, 13
H = H2 // 2          # 128
G4 = 4 * H           # 512
START, STOP = 0, 10
NCORES = 8
BC = B // NCORES     # 32 sequences per core
TOK = BC * T         # 6144 tokens per core, t-major: tok = t*BC + b
KAUG = E + 2         # emb dims + ones row + (1-m) row
KCH = [(0, 128), (128, 256), (256, KAUG)]   # K chunks of augmented GEMM
NCHUNK = 512         # gx GEMM moving free dim
RENORM = 32          # CRF renorm period (P scaled by e^-CCENT keeps fp32 range safe)
CCENT = 3.0
TH = T // 2          # CRF split point: alpha does t=0..TH-1, beta does t=T-1..TH
PB = 32              # beta half partition base (engine ops need 32-aligned starts)
PW = PB + NT         # packed CRF state height (rows 13:32 dead zeros)
NREN = 4             # renorm events: 2 per chain (final interval folded into last log)

FP32 = mybir.dt.float32
BF16 = mybir.dt.bfloat16
I32 = mybir.dt.int32
AF = mybir.ActivationFunctionType
ALU = mybir.AluOpType

_PROGRAM_CACHE = {}
PHASE_LIMIT = 5  # 1=gx only, 2=+lstm, 3=+fc, 4=+crf, 5=all (ablation timing knob)


def _emit(tc, io):
    nc = tc.nc
    ident = io["ident"]; sent = io["sent"]; embtab = io["embtab"]
    waug = io["waug"]; whh = io["whh"]; wfc = io["wfc"]; bfc = io["bfc"]
    transT = io["transT"]; transflat = io["transflat"]; swap26 = io["swap26"]
    aux = io["aux"]; oh = io["oh"]; pc = io["pc"]; out = io["out"]

    import contextlib
    ctx = contextlib.ExitStack()
    with ctx:
        consts = ctx.enter_context(tc.tile_pool(name="consts", bufs=1))

        # ---------- constants into SBUF ----------
        ident_sb = consts.tile([128, 128], BF16)
        nc.sync.dma_start(out=ident_sb[:], in_=ident[:])
        sent_sb = consts.tile([128, TOK // 128], I32)
        nc.sync.dma_start(out=sent_sb[:], in_=sent.rearrange("(c p) -> p c", p=128))
        waug_sb = [consts.tile([k1 - k0, 2 * G4], BF16, name=f"waug{i}") for i, (k0, k1) in enumerate(KCH)]
        for (k0, k1), t_ in zip(KCH, waug_sb):
            nc.sync.dma_start(out=t_[:], in_=waug[k0:k1, :])
        whh_sb = consts.tile([H, 2 * G4], BF16)
        nc.sync.dma_start(out=whh_sb[:], in_=whh[:])
        # fc weights duplicated to 26 out-partitions: rows 13:26 of the fc
        # GEMM replicate rows 0:13 so the beta-order ef can be written
        # without a partition shift.
        wfc_sb = consts.tile([H, 2 * PW], BF16)  # [h_f-half dup45 | h_b-half dup45]
        nc.sync.dma_start(out=wfc_sb[:, 0:PW], in_=wfc[0:H, :])
        nc.sync.dma_start(out=wfc_sb[:, PW:2 * PW], in_=wfc[H:H2, :])
        bfc_sb = consts.tile([PW, 1], FP32)
        nc.sync.dma_start(out=bfc_sb[:], in_=bfc[:])
        transT_sb = consts.tile([NT, NT], FP32)
        nc.sync.dma_start(out=transT_sb[:], in_=transT[:])
        tf_sb = consts.tile([128, 2], FP32)   # transflat split: col0 rows 0:128, col1 rows 0:41
        tfa = transflat[0:128]
        nc.sync.dma_start(out=tf_sb[:, 0:1], in_=bass.AP(tensor=tfa.tensor, offset=tfa.offset, ap=[[1, 128], [1, 1]]))
        tfb = transflat[128:NT * NT + NT]
        nc.sync.dma_start(out=tf_sb[0:54, 1:2], in_=bass.AP(tensor=tfb.tensor, offset=tfb.offset, ap=[[1, 54], [1, 1]]))
        pc_sb = consts.tile([128, 2 * BC], FP32)  # PC chunks side by side
        nc.sync.dma_start(out=pc_sb[:, 0:BC], in_=pc[0:128, :])
        nc.sync.dma_start(out=pc_sb[0:54, BC:2 * BC], in_=pc[128:NT * NT + NT, :])
        ones13_sb = consts.tile([NT, 1], FP32)
        nc.vector.memset(ones13_sb[:], 1.0)
        # P^T = exp(transT), used for the PSTOP column
        pt_sb = consts.tile([NT, NT], FP32)
        nc.scalar.activation(pt_sb[:], transT_sb[:], AF.Exp)
        # block-diagonal fused CRF stationary: lhsT = diag(P^T, P), e^-CCENT
        # centered bf16. One matmul advances BOTH the alpha chain (rows 0:13)
        # and the beta chain (rows 13:26).
        negc_sb = consts.tile([PW, 1], FP32)
        nc.vector.memset(negc_sb[:], -CCENT)
        transN45_sb = consts.tile([PW, NT], FP32)
        nc.sync.dma_start(
            out=transN45_sb[PB:PW, :],
            in_=bass.AP(tensor=transflat.tensor, offset=transflat.offset,
                        ap=[[NT, NT], [1, NT]]),
        )
        pd_sb = consts.tile([PW, PW], BF16)
        nc.vector.memset(pd_sb[:], 0.0)
        nc.scalar.activation(pd_sb[0:NT, 0:NT], transT_sb[:], AF.Exp,
                             bias=negc_sb[0:NT, 0:1])
        nc.scalar.activation(pd_sb[PB:PW, PB:PW], transN45_sb[PB:PW, :],
                             AF.Exp, bias=negc_sb[PB:PW, 0:1])
        swap_sb = consts.tile([PW, PW], BF16)
        nc.sync.dma_start(out=swap_sb[:], in_=swap26[:])
        selt_sb = consts.tile([PW, 1], BF16)
        nc.vector.memset(selt_sb[:], 0.0)
        nc.vector.memset(selt_sb[0:NT, :], 1.0)
        selb_sb = consts.tile([PW, 1], BF16)
        nc.vector.memset(selb_sb[:], 0.0)
        nc.vector.memset(selb_sb[PB:PW, :], 1.0)
        selrt_sb = consts.tile([1, PW], FP32)
        nc.vector.memset(selrt_sb[:], 0.0)
        nc.vector.memset(selrt_sb[:, 0:NT], 1.0)
        selrb_sb = consts.tile([1, PW], FP32)
        nc.vector.memset(selrb_sb[:], 0.0)
        nc.vector.memset(selrb_sb[:, PB:PW], 1.0)
        onesbc_sb = consts.tile([1, BC], FP32)
        nc.vector.memset(onesbc_sb[:], 1.0)

        # ---------- phase 1: gather -> transpose -> gx GEMMs ----------
        # embT and gxd are chunked per 512-token block so downstream deps are
        # per-chunk and the LSTM can start before all of phase 1 finishes.
        hallp = ctx.enter_context(tc.tile_pool(name="hallp", bufs=1))
        h_all = [hallp.tile([H, TOK], BF16, name=f"hall{i}") for i in range(2)]
        late = ctx.enter_context(tc.tile_pool(name="late", bufs=1))
        # ef2: rows 0:13 = exp(feats) in alpha order (block i = t=i); rows
        # 32:45 = exp(feats) in beta order (block i = t=190-i, block 95 = 1);
        # rows 13:32 dead zeros (engine partition starts must be 32-aligned).
        ef2 = late.tile([PW, TOK], BF16)
        nc.vector.memset(ef2[:], 0.0)
        nc.vector.memset(ef2[PB:PW, (TH - 1) * BC:TH * BC], 1.0)
        emstrip = late.tile([NT, (TOK // NCHUNK) * BC], FP32)
        zbuf = late.tile([1, NREN * BC], FP32)
        ohp = ctx.enter_context(tc.tile_pool(name="ohp", bufs=2))
        fctmp = ctx.enter_context(tc.tile_pool(name="fctmp", bufs=2))
        NCH = TOK // NCHUNK  # 12 chunks
        gxdp = tc.alloc_tile_pool(name="gxdp", bufs=1)
        gxd = [[gxdp.tile([128, 4 * NCHUNK], BF16, name=f"gxd{d}_{n}") for n in range(NCH)]
               for d in range(2)]
        embp = tc.alloc_tile_pool(name="embp", bufs=1)
        embT = [[embp.tile([k1 - k0, NCHUNK], BF16, name=f"embT{i}_{n}") for n in range(NCH)]
                for i, (k0, k1) in enumerate(KCH)]

        def chunk_order():
            lo, hi = 0, NCH - 1
            out = []
            while lo <= hi:
                out.append((1, hi)); hi -= 1
                if lo <= hi:
                    out.append((0, lo)); lo += 1
            return out

        for n in range(NCH):
            nc.sync.dma_start(
                out=embT[2][n][E - 256:E - 254, :],
                in_=bass.AP(tensor=aux.tensor, offset=aux.offset + n * NCHUNK,
                            ap=[[TOK, 2], [1, NCHUNK]]),
            )
        gathered = set()

        def emit_gather(c, n):
            embg = gpool.tile([128, E], BF16, name=f"embg{c}", tag="embg")
            nc.gpsimd.indirect_dma_start(
                out=embg[:], out_offset=None, in_=embtab[:],
                in_offset=bass.IndirectOffsetOnAxis(ap=sent_sb[:, c:c + 1], axis=0),
            )
            cc = (c % 4) * 128
            for ki, (k0, k1) in enumerate(KCH):
                kw = min(k1, E) - k0
                tp = tpsum.tile([128, 128], BF16, name=f"tp{c}_{ki}", tag="tp")
                nc.tensor.transpose(tp[0:kw, :], embg[:, k0:k0 + kw], ident_sb[:])
                if (c + ki) % 2 == 0:
                    nc.scalar.copy(embT[ki][n][0:kw, cc:cc + 128], tp[0:kw, :])
                else:
                    nc.vector.tensor_copy(embT[ki][n][0:kw, cc:cc + 128], tp[0:kw, :])

        def emit_gx(dd, n, g):
            mm = gxp.tile([128, NCHUNK], FP32, name=f"gxmm{dd}_{n}_{g}", tag="gxmm")
            for ki, (k0, k1) in enumerate(KCH):
                nc.tensor.matmul(
                    mm[:],
                    waug_sb[ki][:, dd * G4 + g * H: dd * G4 + (g + 1) * H],
                    embT[ki][n][:],
                    start=(ki == 0), stop=(ki == 2),
                )
            dst = gxd[dd][n][:, g * NCHUNK:(g + 1) * NCHUNK]
            if (n + g) % 2 == 0:
                nc.scalar.copy(dst, mm[:])
            else:
                nc.vector.tensor_copy(dst, mm[:])

        def production_items():
            for d, n in chunk_order():
                for c in range(4 * n, 4 * n + 4):
                    if c not in gathered:
                        gathered.add(c)
                        yield ("gather", c, n)
                for dd in (d, 1 - d):
                    for g in range(4):
                        yield ("gx", dd, n, g)

        prod = production_items()

        def emit_items(k):
            for _ in range(k):
                it = next(prod, None)
                if it is None:
                    return
                if it[0] == "gather":
                    emit_gather(it[1], it[2])
                else:
                    emit_gx(it[1], it[2], it[3])

        # ---------- phase 2: LSTM passes (fc GEMM folded into the tail) ----
        do_rest = PHASE_LIMIT >= 2
        hinit = consts.tile([H, BC], BF16)
        nc.vector.memset(hinit[:], 0.0)
        cinit = consts.tile([H, BC], FP32)
        nc.vector.memset(cinit[:], 0.0)

        lpools = {
            "gates": tc.alloc_tile_pool(name="lgates", bufs=2, space="PSUM"),
            "tio": tc.alloc_tile_pool(name="ltio", bufs=2),
            "ab": tc.alloc_tile_pool(name="lab", bufs=2),
            "s": tc.alloc_tile_pool(name="ls", bufs=2),
            "tc": tc.alloc_tile_pool(name="ltc", bufs=2),
        }
        gxp = tc.alloc_tile_pool(name="gxp", bufs=2, space="PSUM")
        tpsum = tc.alloc_tile_pool(name="tpsum", bufs=2, space="PSUM")
        gpool = tc.alloc_tile_pool(name="gather", bufs=12)
        emit_items(24)  # first chunk of each direction up front

        # Cell math via 2*sigmoid(x) = tanh(x/2)+1 and doubled state s = 2c:
        #   a2 = (tio_i+1)*tio_g = 2*sig_i*tanh(g)
        #   b2 = (tio_f+1)*s_prev = 4*sig_f*c_prev
        #   s  = 0.5*b2 + a2 = 2c
        #   tc = tanh(0.5*s) = tanh(c)
        #   h2 = (tio_o+1)*tc = 2h      (Whh and W_fc pre-halved on host)
        # 4 fused STT ops/dir on DVE vs 5 TS/TT ops in the naive form.
        def lstm_step(d, sidx, t, h_prev_ap, s_prev_ap):
            n, toff = t // 16, (t % 16) * BC
            gates = lpools["gates"].tile([H, 4 * BC], FP32, tag=f"gates{d}", name=f"gates{d}")
            gxt = gxd[d][n]
            gx_rhs = bass.AP(
                tensor=gxt.tensor, offset=gxt[:, toff].offset,
                ap=[gxt.ap[0], [NCHUNK, 4], [1, BC]],
            )
            # gx -> PSUM via identity matmul (PE slack; keeps ACT/DVE free)
            nc.tensor.matmul(gates[:], ident_sb[:], gx_rhs, start=True, stop=False,
                             skip_group_check=True)
            for g in range(4):
                nc.tensor.matmul(
                    gates[:, g * BC:(g + 1) * BC],
                    whh_sb[:, d * G4 + g * H: d * G4 + (g + 1) * H],
                    h_prev_ap,
                    start=False, stop=True, skip_group_check=True,
                )
            tio = lpools["tio"].tile([H, 4 * BC], FP32, tag=f"tio{d}", name=f"tio{d}")
            nc.scalar.activation(tio[:], gates[:], AF.Tanh, scale=0.5)
            a2 = lpools["ab"].tile([H, BC], FP32, tag=f"a{d}", name=f"a{d}")
            nc.vector.scalar_tensor_tensor(
                a2[:], tio[:, 0:BC], 1.0, tio[:, 3 * BC:4 * BC], ALU.add, ALU.mult)
            b2 = lpools["ab"].tile([H, BC], FP32, tag=f"b{d}", name=f"b{d}")
            nc.vector.scalar_tensor_tensor(
                b2[:], tio[:, BC:2 * BC], 1.0, s_prev_ap, ALU.add, ALU.mult)
            s_ = lpools["s"].tile([H, BC], FP32, tag=f"s{d}", name=f"s{d}")
            nc.vector.scalar_tensor_tensor(
                s_[:], b2[:], 0.5, a2[:], ALU.mult, ALU.add)
            tc_ = lpools["tc"].tile([H, BC], FP32, tag=f"tc{d}", name=f"tc{d}")
            nc.scalar.activation(tc_[:], s_[:], AF.Tanh, scale=0.5)
            hdst = h_all[d][:, t * BC:(t + 1) * BC]
            nc.vector.scalar_tensor_tensor(
                hdst, tio[:, 2 * BC:3 * BC], 1.0, tc_[:], ALU.add, ALU.mult)
            return hdst, s_[:]

        # fc chunk n is ready once h_f cols (step 16n+15) and h_b cols
        # (step 191-16n) are both written; middle chunks unlock from step 96.
        fcp = None

        def emit_fc(n):
            cols = slice(n * NCHUNK, (n + 1) * NCHUNK)
            oh_ch = ohp.tile([NT, NCHUNK], BF16, tag="ohch", name=f"ohch{n}")
            nc.sync.dma_start(out=oh_ch[:], in_=oh[:, cols])
            mm = fcp.tile([PW, NCHUNK], FP32, tag="fcmm", name=f"fcmm{n}")
            nc.tensor.matmul(mm[:], wfc_sb[:, 0:PW], h_all[0][:, cols],
                             start=True, stop=False)
            nc.tensor.matmul(mm[:], wfc_sb[:, PW:2 * PW], h_all[1][:, cols],
                             start=False, stop=True)
            nc.scalar.activation(ef2[0:NT, cols], mm[0:NT, :], AF.Exp,
                                 bias=bfc_sb[0:NT, 0:1])
            if n >= NCH // 2:
                # beta-order blocks: t -> 190-t (chunk 11 skips t=191)
                nt_ = 16 if n < NCH - 1 else 15
                base = ef2[PB:PW, (190 - 16 * n) * BC:(190 - 16 * n + 1) * BC]
                dstb = bass.AP(tensor=base.tensor, offset=base.offset,
                               ap=[base.ap[0], [-BC, nt_], [1, BC]])
                nc.scalar.activation(dstb, mm[PB:PW, 0:nt_ * BC], AF.Exp,
                                     bias=bfc_sb[PB:PW, 0:1])
            # emit-score contribution: sum_t mm[tag_t] (b_fc part rides the
            # host-built tag-count histogram through the gold matmuls)
            emtmp = fctmp.tile([NT, NCHUNK], FP32, tag="emtmp", name=f"emtmp{n}")
            nc.vector.tensor_tensor(emtmp[:], mm[0:NT, :], oh_ch[:], ALU.mult)
            nc.vector.reduce_sum(
                emstrip[:, n * BC:(n + 1) * BC],
                emtmp[:].rearrange("p (t b) -> p b t", t=16),
                axis=mybir.AxisListType.X)

        FC_REL = 97  # production exhausted well before this step
        fc_sched = sorted(range(NCH), key=lambda n: max(16 * n + 15, T - 1 - 16 * n))
        fc_ready = lambda n, s: s > max(16 * n + 15, T - 1 - 16 * n) + 1

        if do_rest:
            # both directions interleaved: two independent dependency chains
            hp0, cp0 = hinit[:], cinit[:]
            hp1, cp1 = hinit[:], cinit[:]
            fc_next = 0
            for s in range(T):
                emit_items(2 if s % 2 == 0 else 1)
                if s == FC_REL:
                    assert next(prod, None) is None
                    gpool.release()
                    tpsum.release()
                    gxp.release()
                    fcp = tc.alloc_tile_pool(name="fcp", bufs=2, space="PSUM")
                hp0, cp0 = lstm_step(0, s, s, hp0, cp0)
                hp1, cp1 = lstm_step(1, s, T - 1 - s, hp1, cp1)
                if (PHASE_LIMIT >= 3 and s > FC_REL and fc_next < NCH
                        and fc_ready(fc_sched[fc_next], s)):
                    emit_fc(fc_sched[fc_next])
                    fc_next += 1
            for n in fc_sched[fc_next:] if PHASE_LIMIT >= 3 else []:
                emit_fc(n)
        else:
            gpool.release()
            tpsum.release()
            gxp.release()
            fcp = tc.alloc_tile_pool(name="fcp", bufs=2, space="PSUM")

        fcp.release()
        for pname in ["tc", "s", "ab", "tio"]:
            lpools[pname].release()
        lpools["gates"].release()
        embp.release()
        gxdp.release()

        # ---------- phase 4: fused CRF forward ----------
        # alpha (t ascending) and beta (t descending) chains packed into one
        # 26-partition state W = [A; u], advanced by ONE block-diagonal
        # matmul lhsT=diag(P^T, P) plus ONE elementwise ef2 multiply per
        # iteration: 96 iterations instead of 192 chain steps. beta's final
        # projection is its iteration 95 (ef2 bottom block 95 = ones).
        # fwd = log(sum_j A_95[j] * v_96[j]) + sum log Z.
        apool = ctx.enter_context(tc.tile_pool(name="apool", bufs=3))
        crfp = tc.alloc_tile_pool(name="crfp", bufs=2, space="PSUM")
        crfz = tc.alloc_tile_pool(name="crfz", bufs=1, space="PSUM")
        zr = ctx.enter_context(tc.tile_pool(name="zr", bufs=2))
        efx = ctx.enter_context(tc.tile_pool(name="efx", bufs=2))

        # init via swap: tmp26 = [pstop*ef_191 ; onehot(START)], W0 = swap @ tmp26
        assert START == 0  # the onehot memset below needs a 32-aligned row
        tmp45 = apool.tile([PW, BC], BF16, name="tmp45")
        nc.vector.memset(tmp45[:], 0.0)
        nc.vector.tensor_scalar_mul(
            tmp45[0:NT, :], ef2[0:NT, (T - 1) * BC:T * BC], pt_sb[:, STOP:STOP + 1])
        nc.vector.memset(tmp45[PB + START:PB + START + 1, :], 1.0)
        w0m = crfp.tile([PW, BC], FP32, tag="rW", name="rW")
        nc.tensor.matmul(w0m[:], swap_sb[:], tmp45[:], start=True, stop=True)
        W = apool.tile([PW, BC], BF16, tag="W", name="W")
        nc.vector.tensor_copy(W[:], w0m[:])
        pend = None

        def crf_renorm26(top, k, nxt_i):
            sel = selt_sb if top else selb_sb
            selr, selo = (selrt_sb, selrb_sb) if top else (selrb_sb, selrt_sb)
            zrow = crfz.tile([1, BC], FP32, tag="zrow", name="zrow")
            nc.tensor.matmul(zrow[:], sel[:], W[:], start=True, stop=True)
            nc.scalar.copy(zbuf[:, k * BC:(k + 1) * BC], zrow[:])
            zrec = zr.tile([1, BC], FP32, tag="zrec", name="zrec")
            nc.vector.reciprocal(zrec[:], zrow[:])
            zbc = crfz.tile([PW, BC], FP32, tag="zbc", name="zbc")
            nc.tensor.matmul(zbc[:], selr[:], zrec[:], start=True, stop=False)
            nc.tensor.matmul(zbc[:], selo[:], onesbc_sb[:], start=False, stop=True)
            nxt = efx.tile([PW, BC], BF16, tag="efx", name="efx")
            nc.vector.tensor_tensor(
                nxt[:], ef2[:, nxt_i * BC:(nxt_i + 1) * BC], zbc[:], ALU.mult)
            return nxt

        wfin = None
        if PHASE_LIMIT >= 4:
            for i in range(TH):
                r = crfp.tile([PW, BC], FP32, tag="rW", name="rW")
                nc.tensor.matmul(r[:], pd_sb[:], W[:], start=True, stop=True)
                nW = apool.tile([PW, BC], BF16, tag="W", name="W")
                ef_ap = pend[:] if pend is not None else ef2[:, i * BC:(i + 1) * BC]
                pend = None
                nc.vector.tensor_tensor(nW[:], r[:], ef_ap, ALU.mult)
                W = nW
                if i in (30, 62):           # beta renorm (32/64 multiplies done)
                    pend = crf_renorm26(False, 2 + (i - 30) // 32, i + 1)
                elif i in (31, 63):         # alpha renorm
                    pend = crf_renorm26(True, (i - 31) // 32, i + 1)
            sw = crfp.tile([PW, BC], FP32, tag="rW", name="rW")
            nc.tensor.matmul(sw[:], swap_sb[:], W[:], start=True, stop=True)
            wfin = apool.tile([NT, BC], FP32, name="wfin")
            nc.vector.tensor_tensor(wfin[:], W[0:NT, :], sw[0:NT, :], ALU.mult)
        crfz.release()
        crfp.release()

        # ---------- phase 5: finals ----------
        fin = ctx.enter_context(tc.tile_pool(name="fin", bufs=1))
        if PHASE_LIMIT < 5:
            nc.sync.dma_start(out=out[:], in_=zbuf[0:1, 0:1])
            return
        finp = ctx.enter_context(tc.tile_pool(name="finp", bufs=1, space="PSUM"))
        emred = fin.tile([NT, BC], FP32)
        nc.vector.reduce_sum(
            emred[:], emstrip[:].rearrange("p (n b) -> p b n", n=TOK // NCHUNK),
            axis=mybir.AxisListType.X)
        # gold = trans_sc + b_fc tag counts (both via tf/pc) + emit
        gold = finp.tile([1, BC], FP32)
        nc.tensor.matmul(gold[:], tf_sb[:, 0:1], pc_sb[:, 0:BC], start=True, stop=False)
        nc.tensor.matmul(gold[:], tf_sb[0:54, 1:2], pc_sb[0:54, BC:2 * BC], start=False, stop=False)
        nc.tensor.matmul(gold[:], ones13_sb[:], emred[:], start=False, stop=True)
        # fwd score: log(sum_j A*v) + sum_k ln Z_k
        fmm = finp.tile([1, BC], FP32)
        nc.tensor.matmul(fmm[:], ones13_sb[:], wfin[:], start=True, stop=True)
        lnz = fin.tile([1, NREN * BC], FP32)
        nc.scalar.activation(lnz[:], zbuf[:], AF.Ln)
        lsum = fin.tile([1, BC], FP32)
        nc.vector.reduce_sum(
            lsum[:], lnz[:].rearrange("p (k b) -> p b k", k=NREN), axis=mybir.AxisListType.X,
        )
        lfin = fin.tile([1, BC], FP32)
        nc.scalar.activation(lfin[:], fmm[:], AF.Ln)
        fwd = fin.tile([1, BC], FP32)
        nc.vector.tensor_tensor(fwd[:], lfin[:], lsum[:], ALU.add)
        nll = fin.tile([1, BC], FP32)
        nc.vector.tensor_tensor(nll[:], fwd[:], gold[:], ALU.subtract)
        nllc = fin.tile([1, BC], FP32)
        nc.vector.tensor_scalar_add(nllc[:], nll[:], CCENT * T)
        tot = fin.tile([1, 1], FP32)
        nc.vector.reduce_sum(tot[:], nllc[:], axis=mybir.AxisListType.X)
        nc.sync.dma_start(out=out[:], in_=tot[:])


def build_program():
    key = ("nc", PHASE_LIMIT)
    if key in _PROGRAM_CACHE:
        return _PROGRAM_CACHE[key]
    nc = bacc.Bacc("TRN2", target_bir_lowering=False, debug=False, num_devices=NCORES)
    io = {
        "ident": nc.dram_tensor("ident", [128, 128], BF16, kind="ExternalInput").ap(),
        "sent": nc.dram_tensor("sent", [TOK], I32, kind="ExternalInput").ap(),
        "embtab": nc.dram_tensor("embtab", [V, E], BF16, kind="ExternalInput").ap(),
        "waug": nc.dram_tensor("waug", [KAUG, 2 * G4], BF16, kind="ExternalInput").ap(),
        "whh": nc.dram_tensor("whh", [H, 2 * G4], BF16, kind="ExternalInput").ap(),
        "wfc": nc.dram_tensor("wfc", [H2, PW], BF16, kind="ExternalInput").ap(),
        "bfc": nc.dram_tensor("bfc", [PW, 1], FP32, kind="ExternalInput").ap(),
        "transT": nc.dram_tensor("transT", [NT, NT], FP32, kind="ExternalInput").ap(),
        "transflat": nc.dram_tensor("transflat", [NT * NT + NT], FP32, kind="ExternalInput").ap(),
        "aux": nc.dram_tensor("aux", [2, TOK], BF16, kind="ExternalInput").ap(),
        "oh": nc.dram_tensor("oh", [NT, TOK], BF16, kind="ExternalInput").ap(),
        "pc": nc.dram_tensor("pc", [NT * NT + NT, BC], FP32, kind="ExternalInput").ap(),
        "swap26": nc.dram_tensor("swap26", [PW, PW], BF16, kind="ExternalInput").ap(),
        "out": nc.dram_tensor("out", [1, 1], FP32, kind="ExternalOutput").ap(),
    }
    with tile.TileContext(nc) as tc:
        _emit(tc, io)
    nc.compile()
    _PROGRAM_CACHE[key] = nc
    return nc


def host_prep(inputs):
    """Build the 8 per-core input maps (host does only index/layout/dtype prep)."""
    import ml_dtypes
    bf16 = ml_dtypes.bfloat16

    sent = np.asarray(inputs["sentence"]).astype(np.int32)      # [B,T]
    seq_len = np.asarray(inputs["seq_len"]).astype(np.int64)
    tags = np.asarray(inputs["tags"]).astype(np.int64)          # [B,T]
    lens = np.clip(seq_len, 1, T)
    mask = (np.arange(T)[None, :] < lens[:, None]).astype(np.float32)  # [B,T]
    embtab_bf = np.ascontiguousarray(np.asarray(inputs["embedding"], np.float32).astype(bf16))

    def reorder(Wx):  # pytorch gate order i,f,g,o -> i,f,o,g
        i, f, g, o = np.split(np.asarray(Wx, np.float32), 4, 0)
        return np.concatenate([i, f, o, g], 0)

    def build_waug(W_ih, bvec, is_bwd):
        Wr = reorder(W_ih).copy()   # [4H, E]
        br = reorder(np.asarray(bvec, np.float32)[:, None])[:, 0].copy()
        Wr[3 * H:4 * H] *= 2.0      # g-gate preact x2: tanh(0.5*(2x)) = tanh(x)
        br[3 * H:4 * H] *= 2.0
        Waug = np.zeros((KAUG, G4), np.float32)
        Waug[0:E, :] = Wr.T
        Waug[E, :] = br             # ones row -> bias
        # (1-m) row: i,f,o preact mask for BOTH dirs -> h == 0 at padded
        # steps, so fc needs no output mask at all
        Waug[E + 1, 0:3 * H] = -1e9
        return Waug

    waug = np.concatenate(
        [build_waug(inputs["W_ih_f"], inputs["b_f"], False),
         build_waug(inputs["W_ih_b"], inputs["b_b"], True)], axis=1
    ).astype(bf16)                                               # [KAUG, 1024]
    def whh_prep(W):
        # x0.5 compensates the doubled hidden state h2 = 2h the kernel keeps;
        # g-gate rows then x2 on top for the tanh(x/2) sigmoid trick.
        Wr = reorder(W).copy() * 0.5
        Wr[3 * H:4 * H] *= 2.0
        return Wr.T
    whh = np.concatenate(
        [whh_prep(inputs["W_hh_f"]), whh_prep(inputs["W_hh_b"])], axis=1
    ).astype(np.float32).astype(bf16)                            # [H, 1024]
    wfcT = np.asarray(inputs["W_fc"], np.float32).T * 0.5      # [H2,NT], x0.5 for h2
    wfc = np.zeros((H2, PW), np.float32)                        # dup cols 0:13 / 32:45
    wfc[:, 0:NT] = wfcT
    wfc[:, PB:PW] = wfcT
    wfc = np.ascontiguousarray(wfc).astype(bf16)
    bfc1 = np.asarray(inputs["b_fc"], np.float32).reshape(NT, 1)
    bfc = np.zeros((PW, 1), np.float32)
    bfc[0:NT] = bfc1
    bfc[PB:PW] = bfc1
    trans = np.asarray(inputs["transitions"], np.float32)
    transT = np.ascontiguousarray(trans.T)
    # trans values + b_fc appended: gold dot-product covers trans_sc AND the
    # emit b_fc term (via tag-count histogram rows of pc)
    transflat = np.ascontiguousarray(
        np.concatenate([trans.reshape(-1), bfc1[:, 0]]))        # [182]
    swap26 = np.zeros((PW, PW), np.float32)
    for k in range(NT):
        swap26[k, PB + k] = 1.0
        swap26[PB + k, k] = 1.0
    swap26 = swap26.astype(bf16)
    ident = np.eye(128, dtype=np.float32).astype(bf16)

    in_maps = []
    for core in range(NCORES):
        sl = slice(core * BC, (core + 1) * BC)
        s_c, t_c, m_c = sent[sl], tags[sl], mask[sl]             # [BC,T]
        sent_tm = np.ascontiguousarray(s_c.T.reshape(-1)).astype(np.int32)   # tok=t*BC+b
        m_tm = np.ascontiguousarray(m_c.T.reshape(-1)).astype(bf16)
        aux_tm = np.stack([np.ones(TOK, np.float32),
                           1.0 - m_tm.astype(np.float32)]).astype(bf16)
        # one-hot [NT, TOK]
        ohm = np.zeros((NT, TOK), np.float32)
        ttm = t_c.T.reshape(-1)                                  # [TOK]
        ohm[ttm, np.arange(TOK)] = 1.0
        ohm = ohm.astype(bf16)
        # pair-count histogram [169, BC] incl STOP term, plus tag-count
        # rows [169:182] that dot with the appended b_fc values
        pcm = np.zeros((NT * NT + NT, BC), np.float32)
        text = np.concatenate([np.full((BC, 1), START, np.int64), t_c], 1)
        for b_ in range(BC):
            idx = text[b_, 1:] * NT + text[b_, :-1]
            np.add.at(pcm[:, b_], idx, 1.0)
            pcm[STOP * NT + t_c[b_, -1], b_] += 1.0
            np.add.at(pcm[NT * NT:, b_], t_c[b_], 1.0)
        in_maps.append({
            "ident": ident, "sent": sent_tm, "embtab": embtab_bf,
            "waug": waug, "whh": whh, "wfc": wfc, "bfc": bfc,
            "transT": transT, "transflat": transflat,
            "aux": aux_tm, "oh": ohm, "pc": pcm, "swap26": swap26,
        })
    return in_maps


def kernel(**inputs):
    nc = build_program()
    in_maps = host_prep(inputs)
    res = bass_utils.run_bass_kernel_spmd(nc, in_maps, list(range(NCORES)))
    total = sum(float(r["out"][0, 0]) for r in res.results)
    return np.float32(total / B)



# revision 56
# speedup vs baseline: 1.0414x; 1.0414x over previous
"""BiLSTM-CRF NLL loss kernel for 8 Trainium2 NeuronCores (pure batch data-parallel).

Self-contained: hardcodes all shapes. Strategy per core (32 of 256 sequences):
  1. Embedding gather (indirect DMA, bf16 table) -> PE transpose -> augmented
     input-projection GEMM gx = W_aug @ [emb; 1; 1-m] (bias + bwd-mask folded in).
  2. Backward LSTM pass (global t descending), then forward pass. Weights
     stationary bf16 matmuls accumulate onto gx preloaded in PSUM; gates via
     tanh only (sigmoid(x) = 0.5*tanh(x/2)+0.5) so one ACT table set serves
     the whole kernel (exp_and_others: tanh+exp+copy).
  3. Bulk fc GEMM -> feats = mask*(W_fc@h) + b_fc -> exp(feats).
  4. CRF forward in scaled probability space: A <- (P @ A) * ef_t with
     P = exp(trans) stationary on PE; renormalize every 6 steps, log(Z)
     summed in bulk at the end (natural_log set, one switch).
  5. Gold score from host-built integer histograms/one-hots via TTR + tiny
     PE reductions. Output: per-core sum of (fwd - gold); host divides by B.
"""

import numpy as np

import concourse.bacc as bacc
import concourse.bass as bass
import concourse.mybir as mybir
import concourse.tile as tile
from concourse import bass_utils

B, T, E, H2, V, NT = 256, 192, 300, 256, 11626, 13
H = H2 // 2          # 128
G4 = 4 * H           # 512
START, STOP = 0, 10
NCORES = 8
BC = B // NCORES     # 32 sequences per core
TOK = BC * T         # 6144 tokens per core, t-major: tok = t*BC + b
KAUG = E + 2         # emb dims + ones row + (1-m) row
KCH = [(0, 128), (128, 256), (256, KAUG)]   # K chunks of augmented GEMM
NCHUNK = 512         # gx GEMM moving free dim
RENORM = 32          # CRF renorm period (P scaled by e^-CCENT keeps fp32 range safe)
CCENT = 3.0
TH = T // 2          # CRF split point: alpha does t=0..TH-1, beta does t=T-1..TH
PB = 32              # beta half partition base (engine ops need 32-aligned starts)
PW = PB + NT         # packed CRF state height (rows 13:32 dead zeros)
NREN = 4             # renorm events: 2 per chain (final interval folded into last log)

FP32 = mybir.dt.float32
BF16 = mybir.dt.bfloat16
I32 = mybir.dt.int32
AF = mybir.ActivationFunctionType
ALU = mybir.AluOpType

_PROGRAM_CACHE = {}
PHASE_LIMIT = 5  # 1=gx only, 2=+lstm, 3=+fc, 4=+crf, 5=all (ablation timing knob)


def _emit(tc, io):
    nc = tc.nc
    ident = io["ident"]; sent = io["sent"]; embtab = io["embtab"]
    waug = io["waug"]; whh = io["whh"]; wfc = io["wfc"]; bfc = io["bfc"]
    transT = io["transT"]; transflat = io["transflat"]; swap26 = io["swap26"]
    aux = io["aux"]; oh = io["oh"]; pc = io["pc"]; out = io["out"]

    import contextlib
    ctx = contextlib.ExitStack()
    with ctx:
        consts = ctx.enter_context(tc.tile_pool(name="consts", bufs=1))

        # ---------- constants into SBUF ----------
        ident_sb = consts.tile([128, 128], BF16)
        nc.sync.dma_start(out=ident_sb[:], in_=ident[:])
        sent_sb = consts.tile([128, TOK // 128], I32)
        nc.sync.dma_start(out=sent_sb[:], in_=sent.rearrange("(c p) -> p c", p=128))
        waug_sb = [consts.tile([k1 - k0, 2 * G4], BF16, name=f"waug{i}") for i, (k0, k1) in enumerate(KCH)]
        for (k0, k1), t_ in zip(KCH, waug_sb):
            nc.sync.dma_start(out=t_[:], in_=waug[k0:k1, :])
        whh_sb = consts.tile([H, 2 * G4], BF16)
        nc.sync.dma_start(out=whh_sb[:], in_=whh[:])
        # fc weights duplicated to 26 out-partitions: rows 13:26 of the fc
        # GEMM replicate rows 0:13 so the beta-order ef can be written
        # without a partition shift.
        wfc_sb = consts.tile([H, 2 * PW], BF16)  # [h_f-half dup45 | h_b-half dup45]
        nc.sync.dma_start(out=wfc_sb[:, 0:PW], in_=wfc[0:H, :])
        nc.sync.dma_start(out=wfc_sb[:, PW:2 * PW], in_=wfc[H:H2, :])
        bfc_sb = consts.tile([PW, 1], FP32)
        nc.sync.dma_start(out=bfc_sb[:], in_=bfc[:])
        transT_sb = consts.tile([NT, NT], FP32)
        nc.sync.dma_start(out=transT_sb[:], in_=transT[:])
        tf_sb = consts.tile([128, 2], FP32)   # transflat split: col0 rows 0:128, col1 rows 0:41
        tfa = transflat[0:128]
        nc.sync.dma_start(out=tf_sb[:, 0:1], in_=bass.AP(tensor=tfa.tensor, offset=tfa.offset, ap=[[1, 128], [1, 1]]))
        tfb = transflat[128:NT * NT + NT]
        nc.sync.dma_start(out=tf_sb[0:54, 1:2], in_=bass.AP(tensor=tfb.tensor, offset=tfb.offset, ap=[[1, 54], [1, 1]]))
        pc_sb = consts.tile([128, 2 * BC], FP32)  # PC chunks side by side
        nc.sync.dma_start(out=pc_sb[:, 0:BC], in_=pc[0:128, :])
        nc.sync.dma_start(out=pc_sb[0:54, BC:2 * BC], in_=pc[128:NT * NT + NT, :])
        ones13_sb = consts.tile([NT, 1], FP32)
        nc.vector.memset(ones13_sb[:], 1.0)
        # P^T = exp(transT), used for the PSTOP column
        pt_sb = consts.tile([NT, NT], FP32)
        nc.scalar.activation(pt_sb[:], transT_sb[:], AF.Exp)
        # block-diagonal fused CRF stationary: lhsT = diag(P^T, P), e^-CCENT
        # centered bf16. One matmul advances BOTH the alpha chain (rows 0:13)
        # and the beta chain (rows 13:26).
        negc_sb = consts.tile([PW, 1], FP32)
        nc.vector.memset(negc_sb[:], -CCENT)
        transN45_sb = consts.tile([PW, NT], FP32)
        nc.sync.dma_start(
            out=transN45_sb[PB:PW, :],
            in_=bass.AP(tensor=transflat.tensor, offset=transflat.offset,
                        ap=[[NT, NT], [1, NT]]),
        )
        pd_sb = consts.tile([PW, PW], BF16)
        nc.vector.memset(pd_sb[:], 0.0)
        nc.scalar.activation(pd_sb[0:NT, 0:NT], transT_sb[:], AF.Exp,
                             bias=negc_sb[0:NT, 0:1])
        nc.scalar.activation(pd_sb[PB:PW, PB:PW], transN45_sb[PB:PW, :],
                             AF.Exp, bias=negc_sb[PB:PW, 0:1])
        swap_sb = consts.tile([PW, PW], BF16)
        nc.sync.dma_start(out=swap_sb[:], in_=swap26[:])
        selt_sb = consts.tile([PW, 1], BF16)
        nc.vector.memset(selt_sb[:], 0.0)
        nc.vector.memset(selt_sb[0:NT, :], 1.0)
        selb_sb = consts.tile([PW, 1], BF16)
        nc.vector.memset(selb_sb[:], 0.0)
        nc.vector.memset(selb_sb[PB:PW, :], 1.0)
        selrt_sb = consts.tile([1, PW], FP32)
        nc.vector.memset(selrt_sb[:], 0.0)
        nc.vector.memset(selrt_sb[:, 0:NT], 1.0)
        selrb_sb = consts.tile([1, PW], FP32)
        nc.vector.memset(selrb_sb[:], 0.0)
        nc.vector.memset(selrb_sb[:, PB:PW], 1.0)
        onesbc_sb = consts.tile([1, BC], FP32)
        nc.vector.memset(onesbc_sb[:], 1.0)

        # ---------- phase 1: gather -> transpose -> gx GEMMs ----------
        # embT and gxd are chunked per 512-token block so downstream deps are
        # per-chunk and the LSTM can start before all of phase 1 finishes.
        hallp = ctx.enter_context(tc.tile_pool(name="hallp", bufs=1))
        h_all = [hallp.tile([H, TOK], BF16, name=f"hall{i}") for i in range(2)]
        late = ctx.enter_context(tc.tile_pool(name="late", bufs=1))
        # ef2: rows 0:13 = exp(feats) in alpha order (block i = t=i); rows
        # 32:45 = exp(feats) in beta order (block i = t=190-i, block 95 = 1);
        # rows 13:32 dead zeros (engine partition starts must be 32-aligned).
        ef2 = late.tile([PW, TOK], BF16)
        nc.vector.memset(ef2[:], 0.0)
        nc.vector.memset(ef2[PB:PW, (TH - 1) * BC:TH * BC], 1.0)
        emstrip = late.tile([NT, (TOK // NCHUNK) * BC], FP32)
        zbuf = late.tile([1, NREN * BC], FP32)
        ohp = ctx.enter_context(tc.tile_pool(name="ohp", bufs=2))
        fctmp = ctx.enter_context(tc.tile_pool(name="fctmp", bufs=2))
        NCH = TOK // NCHUNK  # 12 chunks
        gxdp = tc.alloc_tile_pool(name="gxdp", bufs=1)
        gxd = [[gxdp.tile([128, 4 * NCHUNK], BF16, name=f"gxd{d}_{n}") for n in range(NCH)]
               for d in range(2)]
        embp = tc.alloc_tile_pool(name="embp", bufs=1)
        embT = [[embp.tile([k1 - k0, NCHUNK], BF16, name=f"embT{i}_{n}") for n in range(NCH)]
                for i, (k0, k1) in enumerate(KCH)]

        def chunk_order():
            lo, hi = 0, NCH - 1
            out = []
            while lo <= hi:
                out.append((1, hi)); hi -= 1
                if lo <= hi:
                    out.append((0, lo)); lo += 1
            return out

        for n in range(NCH):
            nc.sync.dma_start(
                out=embT[2][n][E - 256:E - 254, :],
                in_=bass.AP(tensor=aux.tensor, offset=aux.offset + n * NCHUNK,
                            ap=[[TOK, 2], [1, NCHUNK]]),
            )
        gathered = set()

        def emit_gather(c, n):
            embg = gpool.tile([128, E], BF16, name=f"embg{c}", tag="embg")
            nc.gpsimd.indirect_dma_start(
                out=embg[:], out_offset=None, in_=embtab[:],
                in_offset=bass.IndirectOffsetOnAxis(ap=sent_sb[:, c:c + 1], axis=0),
            )
            cc = (c % 4) * 128
            for ki, (k0, k1) in enumerate(KCH):
                kw = min(k1, E) - k0
                tp = tpsum.tile([128, 128], BF16, name=f"tp{c}_{ki}", tag="tp")
                nc.tensor.transpose(tp[0:kw, :], embg[:, k0:k0 + kw], ident_sb[:])
                if (c + ki) % 2 == 0:
                    nc.scalar.copy(embT[ki][n][0:kw, cc:cc + 128], tp[0:kw, :])
                else:
                    nc.vector.tensor_copy(embT[ki][n][0:kw, cc:cc + 128], tp[0:kw, :])

        def emit_gx(dd, n, g):
            mm = gxp.tile([128, NCHUNK], FP32, name=f"gxmm{dd}_{n}_{g}", tag="gxmm")
            for ki, (k0, k1) in enumerate(KCH):
                nc.tensor.matmul(
                    mm[:],
                    waug_sb[ki][:, dd * G4 + g * H: dd * G4 + (g + 1) * H],
                    embT[ki][n][:],
                    start=(ki == 0), stop=(ki == 2),
                )
            dst = gxd[dd][n][:, g * NCHUNK:(g + 1) * NCHUNK]
            if (n + g) % 2 == 0:
                nc.scalar.copy(dst, mm[:])
            else:
                nc.vector.tensor_copy(dst, mm[:])

        def production_items():
            for d, n in chunk_order():
                for c in range(4 * n, 4 * n + 4):
                    if c not in gathered:
                        gathered.add(c)
                        yield ("gather", c, n)
                for dd in (d, 1 - d):
                    for g in range(4):
                        yield ("gx", dd, n, g)

        prod = production_items()

        def emit_items(k):
            for _ in range(k):
                it = next(prod, None)
                if it is None:
                    return
                if it[0] == "gather":
                    emit_gather(it[1], it[2])
                else:
                    emit_gx(it[1], it[2], it[3])

        # ---------- phase 2: LSTM passes (fc GEMM folded into the tail) ----
        do_rest = PHASE_LIMIT >= 2
        hinit = consts.tile([H, BC], BF16)
        nc.vector.memset(hinit[:], 0.0)
        cinit = consts.tile([H, BC], FP32)
        nc.vector.memset(cinit[:], 0.0)

        lpools = {
            "gates": tc.alloc_tile_pool(name="lgates", bufs=2, space="PSUM"),
            "tio": tc.alloc_tile_pool(name="ltio", bufs=2),
            "ab": tc.alloc_tile_pool(name="lab", bufs=2),
            "s": tc.alloc_tile_pool(name="ls", bufs=2),
            "tc": tc.alloc_tile_pool(name="ltc", bufs=2),
        }
        gxp = tc.alloc_tile_pool(name="gxp", bufs=2, space="PSUM")
        tpsum = tc.alloc_tile_pool(name="tpsum", bufs=2, space="PSUM")
        gpool = tc.alloc_tile_pool(name="gather", bufs=12)
        emit_items(24)  # first chunk of each direction up front

        # Cell math via 2*sigmoid(x) = tanh(x/2)+1 and doubled state s = 2c:
        #   a2 = (tio_i+1)*tio_g = 2*sig_i*tanh(g)
        #   b2 = (tio_f+1)*s_prev = 4*sig_f*c_prev
        #   s  = 0.5*b2 + a2 = 2c
        #   tc = tanh(0.5*s) = tanh(c)
        #   h2 = (tio_o+1)*tc = 2h      (Whh and W_fc pre-halved on host)
        # 4 fused STT ops/dir on DVE vs 5 TS/TT ops in the naive form.
        def lstm_step(d, sidx, t, h_prev_ap, s_prev_ap):
            n, toff = t // 16, (t % 16) * BC
            gates = lpools["gates"].tile([H, 4 * BC], FP32, tag=f"gates{d}", name=f"gates{d}")
            gxt = gxd[d][n]
            gx_rhs = bass.AP(
                tensor=gxt.tensor, offset=gxt[:, toff].offset,
                ap=[gxt.ap[0], [NCHUNK, 4], [1, BC]],
            )
            # gx -> PSUM via identity matmul (PE slack; keeps ACT/DVE free)
            nc.tensor.matmul(gates[:], ident_sb[:], gx_rhs, start=True, stop=False,
                             skip_group_check=True)
            for g in range(4):
                nc.tensor.matmul(
                    gates[:, g * BC:(g + 1) * BC],
                    whh_sb[:, d * G4 + g * H: d * G4 + (g + 1) * H],
                    h_prev_ap,
                    start=False, stop=True, skip_group_check=True,
                )
            tio = lpools["tio"].tile([H, 4 * BC], FP32, tag=f"tio{d}", name=f"tio{d}")
            nc.scalar.activation(tio[:], gates[:], AF.Tanh, scale=0.5)
            a2 = lpools["ab"].tile([H, BC], FP32, tag=f"a{d}", name=f"a{d}")
            nc.vector.scalar_tensor_tensor(
                a2[:], tio[:, 0:BC], 1.0, tio[:, 3 * BC:4 * BC], ALU.add, ALU.mult)
            b2 = lpools["ab"].tile([H, BC], FP32, tag=f"b{d}", name=f"b{d}")
            nc.vector.scalar_tensor_tensor(
                b2[:], tio[:, BC:2 * BC], 1.0, s_prev_ap, ALU.add, ALU.mult)
            s_ = lpools["s"].tile([H, BC], FP32, tag=f"s{d}", name=f"s{d}")
            nc.vector.scalar_tensor_tensor(
                s_[:], b2[:], 0.5, a2[:], ALU.mult, ALU.add)
            tc_ = lpools["tc"].tile([H, BC], FP32, tag=f"tc{d}", name=f"tc{d}")
            nc.scalar.activation(tc_[:], s_[:], AF.Tanh, scale=0.5)
            hdst = h_all[d][:, t * BC:(t + 1) * BC]
            nc.vector.scalar_tensor_tensor(
                hdst, tio[:, 2 * BC:3 * BC], 1.0, tc_[:], ALU.add, ALU.mult)
            return hdst, s_[:]

        # fc chunk n is ready once h_f cols (step 16n+15) and h_b cols
        # (step 191-16n) are both written; middle chunks unlock from step 96.
        fcp = None

        def emit_fc(n):
            cols = slice(n * NCHUNK, (n + 1) * NCHUNK)
            oh_ch = ohp.tile([NT, NCHUNK], BF16, tag="ohch", name=f"ohch{n}")
            nc.sync.dma_start(out=oh_ch[:], in_=oh[:, cols])
            mm = fcp.tile([PW, NCHUNK], FP32, tag="fcmm", name=f"fcmm{n}")
            nc.tensor.matmul(mm[:], wfc_sb[:, 0:PW], h_all[0][:, cols],
                             start=True, stop=False)
            nc.tensor.matmul(mm[:], wfc_sb[:, PW:2 * PW], h_all[1][:, cols],
                             start=False, stop=True)
            nc.scalar.activation(ef2[0:NT, cols], mm[0:NT, :], AF.Exp,
                                 bias=bfc_sb[0:NT, 0:1])
            if n >= NCH // 2:
                # beta-order blocks: t -> 190-t (chunk 11 skips t=191)
                nt_ = 16 if n < NCH - 1 else 15
                base = ef2[PB:PW, (190 - 16 * n) * BC:(190 - 16 * n + 1) * BC]
                dstb = bass.AP(tensor=base.tensor, offset=base.offset,
                               ap=[base.ap[0], [-BC, nt_], [1, BC]])
                nc.scalar.activation(dstb, mm[PB:PW, 0:nt_ * BC], AF.Exp,
                                     bias=bfc_sb[PB:PW, 0:1])
            # emit-score contribution: sum_t mm[tag_t] (b_fc part rides the
            # host-built tag-count histogram through the gold matmuls)
            emtmp = fctmp.tile([NT, NCHUNK], FP32, tag="emtmp", name=f"emtmp{n}")
            nc.vector.tensor_tensor(emtmp[:], mm[0:NT, :], oh_ch[:], ALU.mult)
            nc.vector.reduce_sum(
                emstrip[:, n * BC:(n + 1) * BC],
                emtmp[:].rearrange("p (t b) -> p b t", t=16),
                axis=mybir.AxisListType.X)

        FC_REL = 97  # production exhausted well before this step
        fc_sched = sorted(range(NCH), key=lambda n: max(16 * n + 15, T - 1 - 16 * n))
        fc_ready = lambda n, s: s > max(16 * n + 15, T - 1 - 16 * n) + 1

        if do_rest:
            # both directions interleaved: two independent dependency chains
            hp0, cp0 = hinit[:], cinit[:]
            hp1, cp1 = hinit[:], cinit[:]
            fc_next = 0
            for s in range(T):
                emit_items(2 if s % 2 == 0 else 1)
                if s == FC_REL:
                    assert next(prod, None) is None
                    gpool.release()
                    tpsum.release()
                    gxp.release()
                    fcp = tc.alloc_tile_pool(name="fcp", bufs=2, space="PSUM")
                hp0, cp0 = lstm_step(0, s, s, hp0, cp0)
                hp1, cp1 = lstm_step(1, s, T - 1 - s, hp1, cp1)
                if (PHASE_LIMIT >= 3 and s > FC_REL and fc_next < NCH
                        and fc_ready(fc_sched[fc_next], s)):
                    emit_fc(fc_sched[fc_next])
                    fc_next += 1
            for n in fc_sched[fc_next:] if PHASE_LIMIT >= 3 else []:
                emit_fc(n)
        else:
            gpool.release()
            tpsum.release()
            gxp.release()
            fcp = tc.alloc_tile_pool(name="fcp", bufs=2, space="PSUM")

        fcp.release()
        for pname in ["tc", "s", "ab", "tio"]:
            lpools[pname].release()
        lpools["gates"].release()
        embp.release()
        gxdp.release()

        # ---------- phase 4: fused CRF forward ----------
        # alpha (t ascending) and beta (t descending) chains packed into one
        # 26-partition state W = [A; u], advanced by ONE block-diagonal
        # matmul lhsT=diag(P^T, P) plus ONE elementwise ef2 multiply per
        # iteration: 96 iterations instead of 192 chain steps. beta's final
        # projection is its iteration 95 (ef2 bottom block 95 = ones).
        # fwd = log(sum_j A_95[j] * v_96[j]) + sum log Z.
        apool = ctx.enter_context(tc.tile_pool(name="apool", bufs=3))
        crfp = tc.alloc_tile_pool(name="crfp", bufs=2, space="PSUM")
        crfz = tc.alloc_tile_pool(name="crfz", bufs=1, space="PSUM")
        zr = ctx.enter_context(tc.tile_pool(name="zr", bufs=2))
        efx = ctx.enter_context(tc.tile_pool(name="efx", bufs=2))

        # init via swap: tmp45 = [pstop*ef_191 ; onehot(START)], W0 = swap @ tmp45
        assert START == 0  # the onehot memset below needs a 32-aligned row
        tmp45 = apool.tile([PW, BC], BF16, name="tmp45")
        nc.vector.memset(tmp45[:], 0.0)
        nc.vector.tensor_scalar_mul(
            tmp45[0:NT, :], ef2[0:NT, (T - 1) * BC:T * BC], pt_sb[:, STOP:STOP + 1])
        nc.vector.memset(tmp45[PB + START:PB + START + 1, :], 1.0)
        w0m = crfp.tile([PW, BC], FP32, tag="r0", name="rW0")
        nc.tensor.matmul(w0m[:], swap_sb[:], tmp45[:], start=True, stop=True)
        # two independent batch chains (cols 0:16 / 16:32) hide the
        # MM <-> TT round-trip latency of the fused recursion.
        SB = BC // 2
        Ws, pends = [], [None, None]
        for q in range(2):
            Wq = apool.tile([PW, SB], BF16, tag=f"W{q}", name=f"W{q}")
            nc.vector.tensor_copy(Wq[:], w0m[:, q * SB:(q + 1) * SB])
            Ws.append(Wq)

        def crf_renorm26(q, top, k, nxt_i):
            sel = selt_sb if top else selb_sb
            selr, selo = (selrt_sb, selrb_sb) if top else (selrb_sb, selrt_sb)
            zrow = crfz.tile([1, SB], FP32, tag="zrow", name=f"zrow{q}")
            nc.tensor.matmul(zrow[:], sel[:], Ws[q][:], start=True, stop=True)
            nc.scalar.copy(zbuf[:, k * BC + q * SB:k * BC + (q + 1) * SB], zrow[:])
            zrec = zr.tile([1, SB], FP32, tag=f"zrec{q}", name=f"zrec{q}")
            nc.vector.reciprocal(zrec[:], zrow[:])
            zbc = crfz.tile([PW, SB], FP32, tag="zbc", name=f"zbc{q}")
            nc.tensor.matmul(zbc[:], selr[:], zrec[:], start=True, stop=False)
            nc.tensor.matmul(zbc[:], selo[:], onesbc_sb[:, 0:SB], start=False, stop=True)
            nxt = efx.tile([PW, SB], BF16, tag=f"efx{q}", name=f"efx{q}")
            nc.vector.tensor_tensor(
                nxt[:], ef2[:, nxt_i * BC + q * SB:nxt_i * BC + (q + 1) * SB],
                zbc[:], ALU.mult)
            pends[q] = nxt

        wfin = None
        if PHASE_LIMIT >= 4:
            for i in range(TH):
                for q in range(2):
                    r = crfp.tile([PW, SB], FP32, tag=f"r{q}", name=f"r{q}")
                    nc.tensor.matmul(r[:], pd_sb[:], Ws[q][:], start=True, stop=True)
                    nW = apool.tile([PW, SB], BF16, tag=f"W{q}", name=f"W{q}")
                    ef_ap = (pends[q][:] if pends[q] is not None
                             else ef2[:, i * BC + q * SB:i * BC + (q + 1) * SB])
                    pends[q] = None
                    nc.vector.tensor_tensor(nW[:], r[:], ef_ap, ALU.mult)
                    Ws[q] = nW
                    if i in (30, 62):       # beta renorm (32/64 multiplies done)
                        crf_renorm26(q, False, 2 + (i - 30) // 32, i + 1)
                    elif i in (31, 63):     # alpha renorm
                        crf_renorm26(q, True, (i - 31) // 32, i + 1)
            wfin = apool.tile([NT, BC], FP32, name="wfin")
            for q in range(2):
                sw = crfp.tile([PW, SB], FP32, tag=f"r{q}", name=f"r{q}")
                nc.tensor.matmul(sw[:], swap_sb[:], Ws[q][:], start=True, stop=True)
                nc.vector.tensor_tensor(
                    wfin[:, q * SB:(q + 1) * SB], Ws[q][0:NT, :], sw[0:NT, :], ALU.mult)
        crfz.release()
        crfp.release()

        # ---------- phase 5: finals ----------
        fin = ctx.enter_context(tc.tile_pool(name="fin", bufs=1))
        if PHASE_LIMIT < 5:
            nc.sync.dma_start(out=out[:], in_=zbuf[0:1, 0:1])
            return
        finp = ctx.enter_context(tc.tile_pool(name="finp", bufs=1, space="PSUM"))
        emred = fin.tile([NT, BC], FP32)
        nc.vector.reduce_sum(
            emred[:], emstrip[:].rearrange("p (n b) -> p b n", n=TOK // NCHUNK),
            axis=mybir.AxisListType.X)
        # gold = trans_sc + b_fc tag counts (both via tf/pc) + emit
        gold = finp.tile([1, BC], FP32)
        nc.tensor.matmul(gold[:], tf_sb[:, 0:1], pc_sb[:, 0:BC], start=True, stop=False)
        nc.tensor.matmul(gold[:], tf_sb[0:54, 1:2], pc_sb[0:54, BC:2 * BC], start=False, stop=False)
        nc.tensor.matmul(gold[:], ones13_sb[:], emred[:], start=False, stop=True)
        # fwd score: log(sum_j A*v) + sum_k ln Z_k
        fmm = finp.tile([1, BC], FP32)
        nc.tensor.matmul(fmm[:], ones13_sb[:], wfin[:], start=True, stop=True)
        lnz = fin.tile([1, NREN * BC], FP32)
        nc.scalar.activation(lnz[:], zbuf[:], AF.Ln)
        lsum = fin.tile([1, BC], FP32)
        nc.vector.reduce_sum(
            lsum[:], lnz[:].rearrange("p (k b) -> p b k", k=NREN), axis=mybir.AxisListType.X,
        )
        lfin = fin.tile([1, BC], FP32)
        nc.scalar.activation(lfin[:], fmm[:], AF.Ln)
        fwd = fin.tile([1, BC], FP32)
        nc.vector.tensor_tensor(fwd[:], lfin[:], lsum[:], ALU.add)
        nll = fin.tile([1, BC], FP32)
        nc.vector.tensor_tensor(nll[:], fwd[:], gold[:], ALU.subtract)
        nllc = fin.tile([1, BC], FP32)
        nc.vector.tensor_scalar_add(nllc[:], nll[:], CCENT * T)
        tot = fin.tile([1, 1], FP32)
        nc.vector.reduce_sum(tot[:], nllc[:], axis=mybir.AxisListType.X)
        nc.sync.dma_start(out=out[:], in_=tot[:])


def build_program():
    key = ("nc", PHASE_LIMIT)
    if key in _PROGRAM_CACHE:
        return _PROGRAM_CACHE[key]
    nc = bacc.Bacc("TRN2", target_bir_lowering=False, debug=False, num_devices=NCORES)
    io = {
        "ident": nc.dram_tensor("ident", [128, 128], BF16, kind="ExternalInput").ap(),
        "sent": nc.dram_tensor("sent", [TOK], I32, kind="ExternalInput").ap(),
        "embtab": nc.dram_tensor("embtab", [V, E], BF16, kind="ExternalInput").ap(),
        "waug": nc.dram_tensor("waug", [KAUG, 2 * G4], BF16, kind="ExternalInput").ap(),
        "whh": nc.dram_tensor("whh", [H, 2 * G4], BF16, kind="ExternalInput").ap(),
        "wfc": nc.dram_tensor("wfc", [H2, PW], BF16, kind="ExternalInput").ap(),
        "bfc": nc.dram_tensor("bfc", [PW, 1], FP32, kind="ExternalInput").ap(),
        "transT": nc.dram_tensor("transT", [NT, NT], FP32, kind="ExternalInput").ap(),
        "transflat": nc.dram_tensor("transflat", [NT * NT + NT], FP32, kind="ExternalInput").ap(),
        "aux": nc.dram_tensor("aux", [2, TOK], BF16, kind="ExternalInput").ap(),
        "oh": nc.dram_tensor("oh", [NT, TOK], BF16, kind="ExternalInput").ap(),
        "pc": nc.dram_tensor("pc", [NT * NT + NT, BC], FP32, kind="ExternalInput").ap(),
        "swap26": nc.dram_tensor("swap26", [PW, PW], BF16, kind="ExternalInput").ap(),
        "out": nc.dram_tensor("out", [1, 1], FP32, kind="ExternalOutput").ap(),
    }
    with tile.TileContext(nc) as tc:
        _emit(tc, io)
    nc.compile()
    _PROGRAM_CACHE[key] = nc
    return nc


def host_prep(inputs):
    """Build the 8 per-core input maps (host does only index/layout/dtype prep)."""
    import ml_dtypes
    bf16 = ml_dtypes.bfloat16

    sent = np.asarray(inputs["sentence"]).astype(np.int32)      # [B,T]
    seq_len = np.asarray(inputs["seq_len"]).astype(np.int64)
    tags = np.asarray(inputs["tags"]).astype(np.int64)          # [B,T]
    lens = np.clip(seq_len, 1, T)
    mask = (np.arange(T)[None, :] < lens[:, None]).astype(np.float32)  # [B,T]
    embtab_bf = np.ascontiguousarray(np.asarray(inputs["embedding"], np.float32).astype(bf16))

    def reorder(Wx):  # pytorch gate order i,f,g,o -> i,f,o,g
        i, f, g, o = np.split(np.asarray(Wx, np.float32), 4, 0)
        return np.concatenate([i, f, o, g], 0)

    def build_waug(W_ih, bvec, is_bwd):
        Wr = reorder(W_ih).copy()   # [4H, E]
        br = reorder(np.asarray(bvec, np.float32)[:, None])[:, 0].copy()
        Wr[3 * H:4 * H] *= 2.0      # g-gate preact x2: tanh(0.5*(2x)) = tanh(x)
        br[3 * H:4 * H] *= 2.0
        Waug = np.zeros((KAUG, G4), np.float32)
        Waug[0:E, :] = Wr.T
        Waug[E, :] = br             # ones row -> bias
        # (1-m) row: i,f,o preact mask for BOTH dirs -> h == 0 at padded
        # steps, so fc needs no output mask at all
        Waug[E + 1, 0:3 * H] = -1e9
        return Waug

    waug = np.concatenate(
        [build_waug(inputs["W_ih_f"], inputs["b_f"], False),
         build_waug(inputs["W_ih_b"], inputs["b_b"], True)], axis=1
    ).astype(bf16)                                               # [KAUG, 1024]
    def whh_prep(W):
        # x0.5 compensates the doubled hidden state h2 = 2h the kernel keeps;
        # g-gate rows then x2 on top for the tanh(x/2) sigmoid trick.
        Wr = reorder(W).copy() * 0.5
        Wr[3 * H:4 * H] *= 2.0
        return Wr.T
    whh = np.concatenate(
        [whh_prep(inputs["W_hh_f"]), whh_prep(inputs["W_hh_b"])], axis=1
    ).astype(np.float32).astype(bf16)                            # [H, 1024]
    wfcT = np.asarray(inputs["W_fc"], np.float32).T * 0.5      # [H2,NT], x0.5 for h2
    wfc = np.zeros((H2, PW), np.float32)                        # dup cols 0:13 / 32:45
    wfc[:, 0:NT] = wfcT
    wfc[:, PB:PW] = wfcT
    wfc = np.ascontiguousarray(wfc).astype(bf16)
    bfc1 = np.asarray(inputs["b_fc"], np.float32).reshape(NT, 1)
    bfc = np.zeros((PW, 1), np.float32)
    bfc[0:NT] = bfc1
    bfc[PB:PW] = bfc1
    trans = np.asarray(inputs["transitions"], np.float32)
    transT = np.ascontiguousarray(trans.T)
    # trans values + b_fc appended: gold dot-product covers trans_sc AND the
    # emit b_fc term (via tag-count histogram rows of pc)
    transflat = np.ascontiguousarray(
        np.concatenate([trans.reshape(-1), bfc1[:, 0]]))        # [182]
    swap26 = np.zeros((PW, PW), np.float32)
    for k in range(NT):
        swap26[k, PB + k] = 1.0
        swap26[PB + k, k] = 1.0
    swap26 = swap26.astype(bf16)
    ident = np.eye(128, dtype=np.float32).astype(bf16)

    in_maps = []
    for core in range(NCORES):
        sl = slice(core * BC, (core + 1) * BC)
        s_c, t_c, m_c = sent[sl], tags[sl], mask[sl]             # [BC,T]
        sent_tm = np.ascontiguousarray(s_c.T.reshape(-1)).astype(np.int32)   # tok=t*BC+b
        m_tm = np.ascontiguousarray(m_c.T.reshape(-1)).astype(bf16)
        aux_tm = np.stack([np.ones(TOK, np.float32),
                           1.0 - m_tm.astype(np.float32)]).astype(bf16)
        # one-hot [NT, TOK]
        ohm = np.zeros((NT, TOK), np.float32)
        ttm = t_c.T.reshape(-1)                                  # [TOK]
        ohm[ttm, np.arange(TOK)] = 1.0
        ohm = ohm.astype(bf16)
        # pair-count histogram [169, BC] incl STOP term, plus tag-count
        # rows [169:182] that dot with the appended b_fc values
        pcm = np.zeros((NT * NT + NT, BC), np.float32)
        text = np.concatenate([np.full((BC, 1), START, np.int64), t_c], 1)
        for b_ in range(BC):
            idx = text[b_, 1:] * NT + text[b_, :-1]
            np.add.at(pcm[:, b_], idx, 1.0)
            pcm[STOP * NT + t_c[b_, -1], b_] += 1.0
            np.add.at(pcm[NT * NT:, b_], t_c[b_], 1.0)
        in_maps.append({
            "ident": ident, "sent": sent_tm, "embtab": embtab_bf,
            "waug": waug, "whh": whh, "wfc": wfc, "bfc": bfc,
            "transT": transT, "transflat": transflat,
            "aux": aux_tm, "oh": ohm, "pc": pcm, "swap26": swap26,
        })
    return in_maps


def kernel(**inputs):
    nc = build_program()
    in_maps = host_prep(inputs)
    res = bass_utils.run_bass_kernel_spmd(nc, in_maps, list(range(NCORES)))
    total = sum(float(r["out"][0, 0]) for r in res.results)
    return np.float32(total / B)

